# revision 27
# baseline (speedup 1.0000x reference)
"""AKOrN layer (attention-coupled Kuramoto oscillators) on 8 TRN2 NeuronCores.

Sharding: B*H = 2*4 = 8 (batch, head) pairs -> one pair per core.
Each core computes its head's attention matrix E = exp(scores) entirely in
SBUF (never touches HBM), runs the 5 Kuramoto steps locally, then projects
its own head's contribution to the output: partial^T = Wo_h @ cos(ph)^T,
written transposed ([D, N], bf16). The host unshards by summing the 4
per-head partials of each batch and adding bo - no device collective.

Key performance mechanisms (all HW-measured on this part):
- The PE clock ramps 1.2 -> 2.4 GHz after ~3us of continuous execution and
  drops back after multi-us idles. A low-cost dummy-transpose "keep-warm"
  stream fills every predicted PE idle window so real matmuls run at the
  fast clock.
- The Scalar ACT queue runs [sin-warm | init sins | exp x8 | step sins |
  final cos]: exactly one sin->exp and one exp->sin table switch (hard
  dependency pins the init sins before the first exp).
- HW Sin is only accurate in ~[-pi-0.7, pi+0.7]: phases use the shifted
  representation ph' = phi+pi in [0,2pi), cos(phi) = sin(pi/2-|ph'-pi|)
  with |.| built on the DVE engines (max(x,-x)), keeping Abs off the ACT
  queue. Phases are never wrapped after init: 5 steps drift < 0.7.
- Per-step critical path: ib matmuls -> PSUM evac (Vector+Scalar halves) ->
  PE transposes -> small SBUF copy -> 4-op DVE chain (Vector, with GpSimd
  running the off-path legs) -> s-sin + cos ACTs -> next stationary.

Self-contained: hardcodes all shapes; only imports concourse from the
container's /opt/trn_rl_repo.
"""

import math
import sys

import numpy as np

for _p in ("/opt/trn_rl_repo",):
    if _p not in sys.path:
        sys.path.insert(0, _p)

# Problem constants (from the reference nn.Module)
B, N, D, H, O = 2, 1024, 256, 4, 8
DT, STEPS = 0.1, 5
DK = D // H            # 64 head dim
P = 128                # partitions
NT = N // P            # 8 token tiles
NCORES = 8
SW = 2 * O + 1         # active stationary width per j-tile: [sin | cos | ones] = 17
SWP = 32               # fp8 stationary block stride (dual-fp8 LDWEIGHTS needs 16/32)
ESH = 4.0              # global exp shift: keeps E' = exp(s - ESH) in fp8e4m3
PI = float(np.pi)
TWO_PI = float(2 * np.pi)

_CACHE = {}


def _build_nc():
    import concourse.bacc as bacc
    import concourse.tile as tile
    import concourse.mybir as mybir
    from concourse.masks import make_identity
    from concourse.tile_rust import add_dep_helper

    f32 = mybir.dt.float32
    bf16 = mybir.dt.bfloat16
    f8 = mybir.dt.float8e4
    ALU = mybir.AluOpType
    ACT = mybir.ActivationFunctionType
    DR = mybir.MatmulPerfMode.DoubleRow

    nc = bacc.Bacc(
        "TRN2",
        target_bir_lowering=False,
        debug=False,
        enable_asserts=False,
        num_devices=NCORES,
    )

    # Per-core external inputs (host pre-sliced / transposed)
    xT = nc.dram_tensor("xT", [D, N], bf16, kind="ExternalInput")         # x[b].T (bf16)
    wqT = nc.dram_tensor("wqT", [D, DK], bf16, kind="ExternalInput")      # Wq_h.T
    wkT = nc.dram_tensor("wkT", [D, DK], bf16, kind="ExternalInput")      # Wk_h.T
    wpT = nc.dram_tensor("wpT", [D, O], bf16, kind="ExternalInput")       # Wp_h.T
    bprt = nc.dram_tensor("bprt", [P, NT * O], f32, kind="ExternalInput")  # bp_h+pi tiled
    csdt = nc.dram_tensor("csdt", [P, 1], f32, kind="ExternalInput")      # DT*cs
    dtom = nc.dram_tensor("dtom", [P, NT * O], f32, kind="ExternalInput")  # DT*om tiled
    wob = nc.dram_tensor("wob", [O, D], bf16, kind="ExternalInput")       # Wo.T head rows
    outp = nc.dram_tensor("out", [D, N], bf16, kind="ExternalOutput")     # partial_b,h^T

    with tile.TileContext(nc) as tc:
        with (
            tc.tile_pool(name="const", bufs=1) as const,
            tc.tile_pool(name="data", bufs=1) as data,
            tc.tile_pool(name="work", bufs=2) as work,
            tc.tile_pool(name="ps2", bufs=2, space="PSUM") as ps2,
            tc.tile_pool(name="ps1", bufs=1, space="PSUM") as ps1,
        ):
            # ---------- warm the Sin table set during the input DMA ----------
            with tc.high_priority():
                sin_warm = const.tile([1, 1], f32)
                nc.vector.memset(sin_warm[:, :], 0.0)
                nc.scalar.activation(sin_warm[:, :], sin_warm[:, :], ACT.Sin)

            # ---------- load inputs (spread across engine DMA queues) ----------
            wp_s = const.tile([P, 2 * O], bf16)
            for kt in range(2):
                nc.sync.dma_start(wp_s[:, kt * O:(kt + 1) * O], wpT[kt * P:(kt + 1) * P, :])
            xtb = data.tile([P, 2 * N], bf16)       # x.T, kt-major
            in_engs = [nc.sync, nc.gpsimd]
            for ib in range(2):
                for kt in range(2):
                    in_engs[kt].dma_start(
                        xtb[:, kt * N + ib * 512: kt * N + (ib + 1) * 512],
                        xT[kt * P:(kt + 1) * P, ib * 512:(ib + 1) * 512])

            wq_s = const.tile([P, 2 * DK], bf16)
            wk_s = const.tile([P, 2 * DK], bf16)
            for kt in range(2):
                nc.sync.dma_start(wq_s[:, kt * DK:(kt + 1) * DK], wqT[kt * P:(kt + 1) * P, :])
                nc.gpsimd.dma_start(wk_s[:, kt * DK:(kt + 1) * DK], wkT[kt * P:(kt + 1) * P, :])
            bprt_s = const.tile([P, NT * O], f32)
            nc.gpsimd.dma_start(bprt_s[:, :], bprt[:, :])
            csdt_s = const.tile([P, 1], f32)
            nc.gpsimd.dma_start(csdt_s[:, :], csdt[:, :])
            dtom_s = const.tile([P, NT * O], f32)
            nc.gpsimd.dma_start(dtom_s[:, :], dtom[:, :])
            wob_s = const.tile([O, D], bf16)
            nc.gpsimd.dma_start(wob_s[:, :], wob[:, :])

            ident = const.tile([P, P], f32)
            make_identity(nc, ident[:, :])
            b_mpi = const.tile([P, 1], f32)
            nc.vector.memset(b_mpi[:, :], -PI)
            b_hpi = const.tile([P, 1], f32)
            nc.vector.memset(b_hpi[:, :], PI / 2)
            b_nsh = const.tile([P, 1], f32)
            nc.vector.memset(b_nsh[:, :], -ESH)

            # ---------- PE keep-warm dummy stream ----------
            # The PE clock needs ~3us of continuous execution to reach
            # 2.4 GHz (HW-measured: 512-col matmuls drop 427ns -> 215ns).
            # These dummy transposes fill predicted PE idle windows so the
            # clock ramps early and never falls back mid-kernel.
            warm_ps = ps1.tile([P, SW], f32, tag="warm", bufs=1)

            def warm(n):
                for _ in range(n):
                    nc.tensor.transpose(
                        warm_ps[:, :],
                        ident[0:SW, 0:P],
                        ident[0:SW, 0:SW],
                    )

            warm(24)  # ramp during the input-DMA window

            # ---------- initial phases, natural layout [i_p, it, O] ----------
            # phi0 = x @ Wp.T directly as 16 small matmuls (xT tiles
            # stationary, Wp moving): no [8,N] evac, no PE transposes.
            php = ps1.tile([P, NT * O], f32, tag="pt", bufs=1)
            with tc.high_priority():
                for it in range(NT):
                    for kt in range(2):
                        nc.tensor.matmul(
                            php[:, it * O:(it + 1) * O],
                            lhsT=xtb[:, kt * N + it * P: kt * N + (it + 1) * P],
                            rhs=wp_s[:, kt * O:(kt + 1) * O],
                            start=(kt == 0),
                            stop=(kt == 1),
                        )
            # ph' = wrap(phi0 + bp + pi) into [0, 2pi): one compare-and-
            # correct wrap (|phi0+bp| < 2pi); never wrapped again (5-step
            # drift < 0.7 stays inside HW Sin's accurate range).
            ph = data.tile([P, NT * O], f32)
            wge = work.tile([P, NT * O], f32, tag="wge")
            aab = work.tile([P, NT * O], f32, tag="aab")
            aab3 = aab[:, :].rearrange("p (t o) -> p t o", o=O)

            with tc.high_priority():
                nc.vector.tensor_tensor(ph[:, :], php[:, :], bprt_s[:, :], ALU.add)
                nc.vector.tensor_scalar(wge[:, :], ph[:, :], TWO_PI, None, ALU.is_ge)
                nc.vector.scalar_tensor_tensor(
                    wge[:, :], ph[:, :], 0.0, wge[:, :], ALU.is_lt, ALU.subtract)
                nc.vector.scalar_tensor_tensor(
                    ph[:, :], wge[:, :], TWO_PI, ph[:, :], ALU.mult, ALU.add)
                # |ph - pi| = max(ph-pi, pi-ph) on the DVE engines
                nc.vector.tensor_scalar(aab[:, :], ph[:, :], PI, None, ALU.subtract)
                nc.gpsimd.tensor_scalar(wge[:, :], aab[:, :], -1.0, None, ALU.mult)
                nc.vector.tensor_tensor(aab[:, :], aab[:, :], wge[:, :], ALU.max)

            # ---------- stationary sin/cos/ones tiles ----------
            HBT = NT // 2
            scw_al = data.tile([P, HBT * SWP], f8)
            scw_ah = data.tile([P, HBT * SWP], f8)
            scw_bl = data.tile([P, HBT * SWP], f8)
            scw_bh = data.tile([P, HBT * SWP], f8)
            scws = [(scw_al, scw_ah), (scw_bl, scw_bh)]
            scw3s = [tuple(t[:, :].rearrange("p (t w) -> p t w", w=SWP) for t in pair)
                     for pair in scws]
            for pair in scws:
                for t in pair:
                    for jt in range(HBT):
                        nc.vector.memset(t[:, jt * SWP + 2 * O: jt * SWP + SW], 1.0)

            ph3 = ph[:, :].rearrange("p (t o) -> p t o", o=O)
            HB = NT // 2  # it-tiles per half

            # ---------- init sins (before the exps: one exp table load) ----
            init_acts = []
            with tc.high_priority():
                for hb in range(2):
                    hs = slice(hb * HBT, (hb + 1) * HBT)
                    a1 = nc.scalar.activation(scw3s[0][hb][:, :, 0:O], ph3[:, hs, :],
                                              ACT.Sin, bias=b_mpi[:, :], scale=1.0)
                    a2 = nc.scalar.activation(scw3s[0][hb][:, :, O:2 * O], aab3[:, hs, :],
                                              ACT.Sin, bias=b_hpi[:, :], scale=-1.0)
                    init_acts += [a1, a2]

            # ---------- q/k projections (bf16) ----------
            qt = data.tile([DK, N], bf16)
            ktt = data.tile([DK, N], bf16)
            for dst, w_s in ((qt, wq_s), (ktt, wk_s)):
                for ib in range(2):
                    pq = ps2.tile([DK, 512], f32, tag="pc")
                    for kt in range(2):
                        nc.tensor.matmul(
                            pq[:, :],
                            lhsT=w_s[:, kt * DK:(kt + 1) * DK],
                            rhs=xtb[:, kt * N + ib * 512: kt * N + (ib + 1) * 512],
                            start=(kt == 0),
                            stop=(kt == 1),
                        )
                    nc.vector.tensor_copy(dst[:, ib * 512:(ib + 1) * 512], pq[:, :])

            # ---------- scores + exp -> E^T (fp8, [j_p, jt-major i]) ----------
            etb = data.tile([P, NT * N], f8)
            first_exp = None
            for jt in range(NT):
                psc = ps2.tile([P, N], f32, tag="big")
                for ib in range(2):
                    nc.tensor.matmul(
                        psc[:, ib * 512:(ib + 1) * 512],
                        lhsT=ktt[:, jt * P:(jt + 1) * P],
                        rhs=qt[:, ib * 512:(ib + 1) * 512],
                        start=True,
                        stop=True,
                    )
                # E' = exp(s/sqrt(dk) - ESH): the global shift keeps E' inside
                # fp8e4m3 range; it cancels exactly in coupling = (E'@sc)/(E'@1)
                e_i = nc.scalar.activation(etb[:, jt * N:(jt + 1) * N], psc[:, :],
                                           ACT.Exp, bias=b_nsh[:, :],
                                           scale=1.0 / math.sqrt(DK))
                if first_exp is None:
                    first_exp = e_i
                # keep the PE clock up while the ACT queue drains the exps
                warm(6)
            # the exp block must come after ALL init sins (one table switch)
            add_dep_helper(first_exp.ins, init_acts[-1].ins, sync=False,
                           reason="group ACT ops by table set")

            # ---------- Kuramoto steps ----------
            gfull = data.tile([P, NT * O], f32)
            gfull3 = gfull[:, :].rearrange("p (t o) -> p t o", o=O)
            rinv = data.tile([P, NT], f32)
            # cvg/svg: cos*g and sin*g, pre-scaled off the critical path
            cvg = data.tile([P, NT * O], f32)
            svg = data.tile([P, NT * O], f32)
            cvg3 = cvg[:, :].rearrange("p (t o) -> p t o", o=O)
            svg3 = svg[:, :].rearrange("p (t o) -> p t o", o=O)
            cnat_l = data.tile([P, HB * O], bf16)
            cnat_h = data.tile([P, HB * O], bf16)
            cnats = [cnat_l, cnat_h]

            phd = data.tile([P, NT * O], f32)
            phd3 = phd[:, :].rearrange("p (t o) -> p t o", o=O)
            phdm = data.tile([P, NT * O], f32)    # phd - pi
            phdm3 = phdm[:, :].rearrange("p (t o) -> p t o", o=O)
            phdmn = data.tile([P, NT * O], f32)   # pi - phd
            phdmn3 = phdmn[:, :].rearrange("p (t o) -> p t o", o=O)
            mhm = work.tile([P, NT * O], f32, tag="mhm")
            mhm3 = mhm[:, :].rearrange("p (t o) -> p t o", o=O)
            # SBUF copy of the transposed coupling sums (GpSimd cannot read
            # PSUM; also shortens the Vector PSUM-port pressure)
            pts = data.tile([P, NT * SW], f32)
            pts3 = pts[:, :].rearrange("p (t w) -> p t w", w=SW)

            def half_update(step, hb, pt3, scw3, scw3_nxt):
                # coupling -> phase update -> next stationary sin/cos
                hs = slice(hb * HB, (hb + 1) * HB)
                nc.vector.tensor_copy(pts3[:, hs, :], pt3[:, hs, :])
                es_v = pts3[:, hs, 0:O]
                ec_v = pts3[:, hs, O:2 * O]
                sv = scw3[hb][:, :, 0:O]
                cv = scw3[hb][:, :, O:2 * O]
                ph_h = ph3[:, hs, :]
                t1 = work.tile([P, HB * O], f32, tag=f"t1{hb}", name=f"t1{hb}")
                t13 = t1[:, :].rearrange("p (t o) -> p t o", o=O)
                t2 = work.tile([P, HB * O], f32, tag=f"t2{hb}", name=f"t2{hb}")
                t23 = t2[:, :].rearrange("p (t o) -> p t o", o=O)
                if step == 0:
                    # g = DT*cs / (E'@1), constant across steps
                    nc.vector.reciprocal(rinv[:, hs, None], pts3[:, hs, 2 * O:SW])
                    nc.vector.tensor_scalar(
                        gfull3[:, hs, :],
                        rinv[:, hs, None].to_broadcast((P, HB, O)),
                        csdt_s[:, :], None, ALU.mult,
                    )
                    nc.vector.tensor_tensor(t13, cv, es_v, ALU.mult)
                    nc.gpsimd.tensor_tensor(t23, sv, ec_v, ALU.mult)
                    nc.vector.tensor_tensor(t13, t13, t23, ALU.subtract)
                    nc.vector.tensor_tensor(t13, t13, gfull3[:, hs, :], ALU.mult)
                else:
                    nc.vector.tensor_tensor(t13, cvg3[:, hs, :], es_v, ALU.mult)
                    nc.gpsimd.tensor_tensor(t23, svg3[:, hs, :], ec_v, ALU.mult)
                    nc.vector.tensor_tensor(t13, t13, t23, ALU.subtract)
                nc.gpsimd.tensor_tensor(mhm3[:, hs, :], phdmn3[:, hs, :], t13,
                                        ALU.subtract)
                nc.vector.tensor_tensor(ph_h, t13, phd3[:, hs, :], ALU.add)
                nc.vector.tensor_tensor(aab3[:, hs, :], t13, phdm3[:, hs, :], ALU.add)
                nc.vector.tensor_tensor(aab3[:, hs, :], aab3[:, hs, :],
                                        mhm3[:, hs, :], ALU.max)
                if step < STEPS - 1:
                    nc.scalar.activation(scw3_nxt[hb][:, :, 0:O], ph_h,
                                         ACT.Sin, bias=b_mpi[:, :], scale=1.0)
                    nc.scalar.activation(scw3_nxt[hb][:, :, O:2 * O], aab3[:, hs, :],
                                         ACT.Sin, bias=b_hpi[:, :], scale=-1.0)
                else:
                    # final sig = cos(phases), per half
                    cn3 = cnats[hb][:, :].rearrange("p (t o) -> p t o", o=O)
                    nc.scalar.activation(cn3, aab3[:, hs, :],
                                         ACT.Sin, bias=b_hpi[:, :], scale=-1.0)

            for step in range(STEPS):
                scw_pair = scws[step % 2]
                scw3 = scw3s[step % 2]
                scw3_nxt = scw3s[(step + 1) % 2]
                last = step == STEPS - 1
                # off-critical-path precompute (overlaps the matmul stream)
                nc.vector.tensor_tensor(phd[:, :], ph[:, :], dtom_s[:, :], ALU.add)
                nc.vector.tensor_scalar(phdm[:, :], phd[:, :], PI, None, ALU.subtract)
                nc.gpsimd.tensor_scalar(phdmn[:, :], phdm[:, :], -1.0, None, ALU.mult)
                if step > 0:
                    for hb in range(2):
                        hs = slice(hb * HB, (hb + 1) * HB)
                        nc.gpsimd.tensor_tensor(cvg3[:, hs, :], scw3[hb][:, :, O:2 * O],
                                                gfull3[:, hs, :], ALU.mult)
                        nc.gpsimd.tensor_tensor(svg3[:, hs, :], scw3[hb][:, :, 0:O],
                                                gfull3[:, hs, :], ALU.mult)

                def dr_pair(pr):
                    # [j_p, 2 j-subtiles, SW] fp8 stationary for DoubleRow
                    t = scw_pair[pr // (HBT // 2)]
                    j = pr % (HBT // 2)
                    return t[:, j * 2 * SWP:(j + 1) * 2 * SWP].rearrange(
                        "p (s w) -> p s w", s=2)[:, :, 0:SW]

                def dr_rhs(pr, ib):
                    # [j_p, 2 j-subtiles, 512] fp8 moving view of E^T
                    return etb[:, pr * 2 * N: (pr + 1) * 2 * N].rearrange(
                        "p (s i) -> p s i", s=2)[:, :, ib * 512:(ib + 1) * 512]

                NPR = NT // 2
                pt = ps1.tile([P, NT * SW], f32, tag="pt", bufs=1)
                pt3 = pt[:, 0:NT * SW].rearrange("p (t w) -> p t w", w=SW)

                # the last step has no next-step matmuls to feed: process the
                # hb1 half FIRST so its (longer) output tail starts earlier
                ib_order = (1, 0) if last else (0, 1)
                pcs = {}
                for ii, ib in enumerate(ib_order):
                    pc = ps2.tile([SW, 512], f32, tag="pc", name=f"pc{ib}")
                    pcs[ib] = pc
                    for pr in range(NPR):
                        nc.tensor.matmul(
                            pc[:, :],
                            lhsT=dr_pair(pr),
                            rhs=dr_rhs(pr, ib),
                            start=(pr == 0),
                            stop=(pr == NPR - 1),
                            perf_mode=DR,
                        )
                        if ii == 1 and pr == 0:
                            hbf = ib_order[0]
                            for itl in range(HB):
                                nc.tensor.transpose(
                                    pt[:, (hbf * HB + itl) * SW:(hbf * HB + itl + 1) * SW],
                                    ce[:, itl * P:(itl + 1) * P],
                                    ident[0:SW, 0:SW],
                                )
                        if ii == 1 and pr == 1:
                            with tc.high_priority(offset=24):
                                half_update(step, ib_order[0], pt3, scw3, scw3_nxt)
                    # PSUM evac split Vector + Scalar (copy is a filler in
                    # every ACT table set: no table switch)
                    ce = work.tile([SW, 512], f32, tag=f"ce{ib}", name=f"ce{ib}")
                    nc.vector.tensor_copy(ce[:, 0:256], pc[:, 0:256])
                    nc.scalar.copy(ce[:, 256:512], pc[:, 256:512])
                hbs = ib_order[1]
                for itl in range(HB):
                    nc.tensor.transpose(
                        pt[:, (hbs * HB + itl) * SW:(hbs * HB + itl + 1) * SW],
                        ce[:, itl * P:(itl + 1) * P],
                        ident[0:SW, 0:SW],
                    )
                # keep the PE clock up through the chain/ACT window
                warm(10 if not last else 4)
                with tc.high_priority(offset=24):
                    half_update(step, hbs, pt3, scw3, scw3_nxt)

            # ---------- partial output projection (per token-half) ----------
            # sig_h^T tiles [O, 512] via PE transpose, then
            # partial^T[d, i] = sum_o Wo_h.T[o, d] * sig_h^T[o, i].
            identb = const.tile([P, P], bf16)
            nc.vector.tensor_copy(identb[:, :], ident[:, :])
            out_engs = [nc.sync, nc.scalar, nc.gpsimd, nc.sync]
            for oi, hb in enumerate((1, 0)):
                psth = ps2.tile([O, 512], bf16, tag="pc", name=f"psth{hb}")
                for itl in range(HB):
                    nc.tensor.transpose(
                        psth[:, itl * P:(itl + 1) * P],
                        cnats[hb][:, itl * O:(itl + 1) * O],
                        identb[:, :],
                    )
                sgth = work.tile([O, 512], bf16, tag="sgt2", name=f"sgth{hb}")
                nc.vector.tensor_copy(sgth[:, :], psth[:, :])
                for dtile in range(2):
                    po = ps2.tile([P, 512], f32, tag="big", name=f"po{hb}{dtile}")
                    nc.tensor.matmul(
                        po[:, :],
                        lhsT=wob_s[:, dtile * P:(dtile + 1) * P],
                        rhs=sgth[:, :],
                        start=True, stop=True)
                    ot = work.tile([P, 512], bf16, tag="ot", name=f"ot{hb}{dtile}")
                    nc.vector.tensor_copy(ot[:, 0:256], po[:, 0:256])
                    nc.scalar.copy(ot[:, 256:512], po[:, 256:512])
                    out_engs[oi * 2 + dtile].dma_start(
                        outp[dtile * P:(dtile + 1) * P, hb * 512:(hb + 1) * 512],
                        ot[:, :])

    nc.compile()
    return nc


def get_nc():
    if "nc" not in _CACHE:
        _CACHE["nc"] = _build_nc()
    return _CACHE["nc"]


def make_in_maps(x, Wq, Wk, Wp, bp, Wo, bo, omega, coupling_scale):
    import concourse.mybir as mybir

    bf16 = mybir.dt.np(mybir.dt.bfloat16)
    x = np.asarray(x, np.float32)
    Wq = np.asarray(Wq, np.float32)
    Wk = np.asarray(Wk, np.float32)
    Wp = np.asarray(Wp, np.float32)
    bp = np.asarray(bp, np.float32)
    Wo = np.asarray(Wo, np.float32)
    bo = np.asarray(bo, np.float32)
    omega = np.asarray(omega, np.float32)
    cs = float(np.asarray(coupling_scale, np.float32))

    csdt_full = np.full((P, 1), DT * cs, np.float32)

    in_maps = []
    for c in range(NCORES):
        b, h = c // H, c % H
        in_maps.append({
            "xT": np.ascontiguousarray(x[b].T).astype(bf16),
            "wqT": np.ascontiguousarray(Wq[h * DK:(h + 1) * DK, :].T).astype(bf16),
            "wkT": np.ascontiguousarray(Wk[h * DK:(h + 1) * DK, :].T).astype(bf16),
            "wpT": np.ascontiguousarray(Wp[h * O:(h + 1) * O, :].T).astype(bf16),
            "bprt": np.ascontiguousarray(
                np.tile((bp[h * O:(h + 1) * O] + np.pi)[None, :], (P, NT)), np.float32),
            "csdt": csdt_full,
            "dtom": np.ascontiguousarray(
                np.tile((DT * omega[h])[None, :], (P, NT)), np.float32),
            "wob": np.ascontiguousarray(Wo.T[h * O:(h + 1) * O, :]).astype(bf16),
        })
    return in_maps


def run_on_hw(in_maps, trace=False):
    from concourse.bass_utils import run_bass_kernel_spmd

    nc = get_nc()
    return run_bass_kernel_spmd(nc, in_maps, core_ids=list(range(NCORES)), trace=trace)


def assemble(outs, bo):
    """Unshard: sum the 4 per-head partials of each batch, add bo."""
    bo = np.asarray(bo, np.float32)
    full = np.empty((B, N, D), np.float32)
    for b in range(B):
        acc = np.asarray(outs[b * H], np.float32)
        for h in range(1, H):
            acc = acc + np.asarray(outs[b * H + h], np.float32)
        full[b] = acc.T + bo[None, :]
    return full


def kernel(x, Wq, Wk, Wp, bp, Wo, bo, omega, coupling_scale):
    in_maps = make_in_maps(x, Wq, Wk, Wp, bp, Wo, bo, omega, coupling_scale)
    res = run_on_hw(in_maps, trace=False)
    return assemble([res.results[c]["out"] for c in range(NCORES)], bo)


# revision 34
# speedup vs baseline: 1.1638x; 1.1638x over previous
"""AKOrN layer (attention-coupled Kuramoto oscillators) on 8 TRN2 NeuronCores.

Sharding: B*H = 2*4 = 8 (batch, head) pairs -> one pair per core.
Each core computes its head's attention matrix E = exp(scores) entirely in
SBUF (never touches HBM), runs the 5 Kuramoto steps locally, then projects
its own head's contribution to the output: partial^T = Wo_h @ cos(ph)^T,
written transposed ([D, N], bf16). The host unshards by summing the 4
per-head partials of each batch and adding bo - no device collective.

Key performance mechanisms (all HW-measured on this part):
- The PE clock ramps 1.2 -> 2.4 GHz after ~3us of continuous execution and
  drops back after multi-us idles. A low-cost dummy-transpose "keep-warm"
  stream fills every predicted PE idle window so real matmuls run at the
  fast clock.
- The Scalar ACT queue runs [sin-warm | init sins | exp x8 | step sins |
  final cos]: exactly one sin->exp and one exp->sin table switch (hard
  dependency pins the init sins before the first exp).
- HW Sin is only accurate in ~[-pi-0.7, pi+0.7]: phases use the shifted
  representation ph' = phi+pi in [0,2pi), cos(phi) = sin(pi/2-|ph'-pi|)
  with |.| built on the DVE engines (max(x,-x)), keeping Abs off the ACT
  queue. Phases are never wrapped after init: 5 steps drift < 0.7.
- Per-step critical path: ib matmuls -> PSUM evac (Vector+Scalar halves) ->
  PE transposes -> small SBUF copy -> 4-op DVE chain (Vector, with GpSimd
  running the off-path legs) -> s-sin + cos ACTs -> next stationary.

Self-contained: hardcodes all shapes; only imports concourse from the
container's /opt/trn_rl_repo.
"""

import math
import sys

import numpy as np

for _p in ("/opt/trn_rl_repo",):
    if _p not in sys.path:
        sys.path.insert(0, _p)

# Problem constants (from the reference nn.Module)
B, N, D, H, O = 2, 1024, 256, 4, 8
DT, STEPS = 0.1, 5
DK = D // H            # 64 head dim
P = 128                # partitions
NT = N // P            # 8 token tiles
NCORES = 8
SW = 2 * O + 1         # active stationary width per j-tile: [sin | cos | ones] = 17
SWP = 32               # fp8 stationary block stride (dual-fp8 LDWEIGHTS needs 16/32)
ESH = 4.0              # global exp shift: keeps E' = exp(s - ESH) in fp8e4m3
PI = float(np.pi)
TWO_PI = float(2 * np.pi)

_CACHE = {}


def _build_nc():
    import concourse.bacc as bacc
    import concourse.tile as tile
    import concourse.mybir as mybir
    from concourse.masks import make_identity
    from concourse.tile_rust import add_dep_helper

    f32 = mybir.dt.float32
    bf16 = mybir.dt.bfloat16
    f8 = mybir.dt.float8e4
    ALU = mybir.AluOpType
    ACT = mybir.ActivationFunctionType
    DR = mybir.MatmulPerfMode.DoubleRow

    nc = bacc.Bacc(
        "TRN2",
        target_bir_lowering=False,
        debug=False,
        enable_asserts=False,
        num_devices=NCORES,
    )

    # Per-core external inputs (host pre-sliced / transposed)
    xT = nc.dram_tensor("xT", [D, N], bf16, kind="ExternalInput")         # x[b].T (bf16)
    wqT = nc.dram_tensor("wqT", [D, DK], bf16, kind="ExternalInput")      # Wq_h.T
    wkT = nc.dram_tensor("wkT", [D, DK], bf16, kind="ExternalInput")      # Wk_h.T
    wpT = nc.dram_tensor("wpT", [D, O], bf16, kind="ExternalInput")       # Wp_h.T
    bprt = nc.dram_tensor("bprt", [P, NT * O], f32, kind="ExternalInput")  # bp_h+pi tiled
    csdt = nc.dram_tensor("csdt", [P, 1], f32, kind="ExternalInput")      # DT*cs
    dtom = nc.dram_tensor("dtom", [P, NT * O], f32, kind="ExternalInput")  # DT*om tiled
    wob = nc.dram_tensor("wob", [O, D], bf16, kind="ExternalInput")       # Wo.T head rows
    outp = nc.dram_tensor("out", [D, N], bf16, kind="ExternalOutput")     # partial_b,h^T

    with tile.TileContext(nc) as tc:
        with (
            tc.tile_pool(name="const", bufs=1) as const,
            tc.tile_pool(name="data", bufs=1) as data,
            tc.tile_pool(name="work", bufs=2) as work,
            tc.tile_pool(name="ps2", bufs=2, space="PSUM") as ps2,
            tc.tile_pool(name="ps1", bufs=1, space="PSUM") as ps1,
        ):
            # ---------- warm the Sin table set during the input DMA ----------
            with tc.high_priority():
                sin_warm = const.tile([1, 1], f32)
                nc.vector.memset(sin_warm[:, :], 0.0)
                nc.scalar.activation(sin_warm[:, :], sin_warm[:, :], ACT.Sin)

            # ---------- load inputs (spread across engine DMA queues) ----------
            wp_s = const.tile([P, 2 * O], bf16)
            for kt in range(2):
                nc.sync.dma_start(wp_s[:, kt * O:(kt + 1) * O], wpT[kt * P:(kt + 1) * P, :])
            xtb = data.tile([P, 2 * N], bf16)       # x.T, kt-major
            in_engs = [nc.sync, nc.gpsimd]
            for ib in range(2):
                for kt in range(2):
                    in_engs[kt].dma_start(
                        xtb[:, kt * N + ib * 512: kt * N + (ib + 1) * 512],
                        xT[kt * P:(kt + 1) * P, ib * 512:(ib + 1) * 512])

            wq_s = const.tile([P, 2 * DK], bf16)
            wk_s = const.tile([P, 2 * DK], bf16)
            for kt in range(2):
                nc.sync.dma_start(wq_s[:, kt * DK:(kt + 1) * DK], wqT[kt * P:(kt + 1) * P, :])
                nc.gpsimd.dma_start(wk_s[:, kt * DK:(kt + 1) * DK], wkT[kt * P:(kt + 1) * P, :])
            bprt_s = const.tile([P, NT * O], f32)
            nc.gpsimd.dma_start(bprt_s[:, :], bprt[:, :])
            csdt_s = const.tile([P, 1], f32)
            nc.gpsimd.dma_start(csdt_s[:, :], csdt[:, :])
            dtom_s = const.tile([P, NT * O], f32)
            nc.gpsimd.dma_start(dtom_s[:, :], dtom[:, :])
            wob_s = const.tile([O, D], bf16)
            nc.gpsimd.dma_start(wob_s[:, :], wob[:, :])

            ident = const.tile([P, P], f32)
            make_identity(nc, ident[:, :])
            b_mpi = const.tile([P, 1], f32)
            nc.vector.memset(b_mpi[:, :], -PI)
            b_hpi = const.tile([P, 1], f32)
            nc.vector.memset(b_hpi[:, :], PI / 2)
            b_nsh = const.tile([P, 1], f32)
            nc.vector.memset(b_nsh[:, :], -ESH)

            # ---------- PE keep-warm dummy stream ----------
            # The PE clock needs ~3us of continuous execution to reach
            # 2.4 GHz (HW-measured: 512-col matmuls drop 427ns -> 215ns).
            # These dummy transposes fill predicted PE idle windows so the
            # clock ramps early and never falls back mid-kernel.
            warm_ps = ps1.tile([P, 512], f32, tag="warm", bufs=1)

            def warm(n):
                for _ in range(n):
                    nc.tensor.transpose(
                        warm_ps[:, 0:SW],
                        ident[0:SW, 0:P],
                        ident[0:SW, 0:SW],
                    )

            warm(12)  # ramp during the input-DMA window

            def warm_heavy(n, after, lhsT, rhs):
                # real-shaped DR matmuls into a scratch bank: heavy enough to
                # hold the PE boost clock, dep-pinned so the list scheduler
                # cannot hoist them out of their intended idle window
                for _ in range(n):
                    wmm = nc.tensor.matmul(
                        warm_ps[0:SW, :], lhsT=lhsT, rhs=rhs,
                        start=True, stop=True, perf_mode=DR)
                    add_dep_helper(wmm.ins, after.ins, sync=False,
                                   reason="PE keep-warm pin")

            # ---------- initial phases, natural layout [i_p, it, O] ----------
            # phi0 = x @ Wp.T directly as 16 small matmuls (xT tiles
            # stationary, Wp moving): no [8,N] evac, no PE transposes.
            php = ps1.tile([P, NT * O], f32, tag="pt", bufs=1)
            with tc.high_priority():
                for it in range(NT):
                    for kt in range(2):
                        nc.tensor.matmul(
                            php[:, it * O:(it + 1) * O],
                            lhsT=xtb[:, kt * N + it * P: kt * N + (it + 1) * P],
                            rhs=wp_s[:, kt * O:(kt + 1) * O],
                            start=(kt == 0),
                            stop=(kt == 1),
                        )
            # ph' = wrap(phi0 + bp + pi) into [0, 2pi): one compare-and-
            # correct wrap (|phi0+bp| < 2pi); never wrapped again (5-step
            # drift < 0.7 stays inside HW Sin's accurate range).
            ph = data.tile([P, NT * O], f32)
            wge = work.tile([P, NT * O], f32, tag="wge")
            aab = work.tile([P, NT * O], f32, tag="aab")
            aab3 = aab[:, :].rearrange("p (t o) -> p t o", o=O)

            with tc.high_priority():
                nc.vector.tensor_tensor(ph[:, :], php[:, :], bprt_s[:, :], ALU.add)
                nc.vector.tensor_scalar(wge[:, :], ph[:, :], TWO_PI, None, ALU.is_ge)
                nc.vector.scalar_tensor_tensor(
                    wge[:, :], ph[:, :], 0.0, wge[:, :], ALU.is_lt, ALU.subtract)
                nc.vector.scalar_tensor_tensor(
                    ph[:, :], wge[:, :], TWO_PI, ph[:, :], ALU.mult, ALU.add)
                # |ph - pi| = max(ph-pi, pi-ph) on the DVE engines
                nc.vector.tensor_scalar(aab[:, :], ph[:, :], PI, None, ALU.subtract)
                nc.vector.tensor_scalar(wge[:, :], aab[:, :], -1.0, None, ALU.mult)
                aab_done = nc.vector.tensor_tensor(aab[:, :], aab[:, :], wge[:, :],
                                                   ALU.max)

            # ---------- stationary sin/cos/ones tiles ----------
            HBT = NT // 2
            scw_al = data.tile([P, HBT * SWP], f8)
            scw_ah = data.tile([P, HBT * SWP], f8)
            scw_bl = data.tile([P, HBT * SWP], f8)
            scw_bh = data.tile([P, HBT * SWP], f8)
            scws = [(scw_al, scw_ah), (scw_bl, scw_bh)]
            scw3s = [tuple(t[:, :].rearrange("p (t w) -> p t w", w=SWP) for t in pair)
                     for pair in scws]
            for pair in scws:
                for t in pair:
                    for jt in range(HBT):
                        nc.gpsimd.memset(t[:, jt * SWP + 2 * O: jt * SWP + SW], 1.0)

            ph3 = ph[:, :].rearrange("p (t o) -> p t o", o=O)
            HB = NT // 2  # it-tiles per half

            # ---------- init sins (before the exps: one exp table load) ----
            init_acts = []
            with tc.high_priority():
                for hb in range(2):
                    hs = slice(hb * HBT, (hb + 1) * HBT)
                    a1 = nc.scalar.activation(scw3s[0][hb][:, :, 0:O], ph3[:, hs, :],
                                              ACT.Sin, bias=b_mpi[:, :], scale=1.0)
                    a2 = nc.scalar.activation(scw3s[0][hb][:, :, O:2 * O], aab3[:, hs, :],
                                              ACT.Sin, bias=b_hpi[:, :], scale=-1.0)
                    init_acts += [a1, a2]

            # ---------- q/k projections (bf16) ----------
            qt = data.tile([DK, N], bf16)
            ktt = data.tile([DK, N], bf16)
            for dst, w_s in ((qt, wq_s), (ktt, wk_s)):
                for ib in range(2):
                    pq = ps2.tile([DK, 512], f32, tag="pc")
                    for kt in range(2):
                        nc.tensor.matmul(
                            pq[:, :],
                            lhsT=w_s[:, kt * DK:(kt + 1) * DK],
                            rhs=xtb[:, kt * N + ib * 512: kt * N + (ib + 1) * 512],
                            start=(kt == 0),
                            stop=(kt == 1),
                        )
                    ev = nc.vector.tensor_copy(dst[:, ib * 512:(ib + 1) * 512], pq[:, :])
                    # keep the Vector queue clear for the init-phase chain
                    # (these evacs gate only the scores, which wait on the
                    # exp pipeline anyway)
                    add_dep_helper(ev.ins, aab_done.ins, sync=False,
                                   reason="init chain before q/k evac")

            # ---------- scores + exp -> E^T (fp8, [j_p, jt-major i]) ----------
            etb = data.tile([P, NT * N], f8)
            first_exp = None
            for jt in range(NT):
                psc = ps2.tile([P, N], f32, tag="big")
                for ib in range(2):
                    nc.tensor.matmul(
                        psc[:, ib * 512:(ib + 1) * 512],
                        lhsT=ktt[:, jt * P:(jt + 1) * P],
                        rhs=qt[:, ib * 512:(ib + 1) * 512],
                        start=True,
                        stop=True,
                    )
                # E' = exp(s/sqrt(dk) - ESH): the global shift keeps E' inside
                # fp8e4m3 range; it cancels exactly in coupling = (E'@sc)/(E'@1)
                e_i = nc.scalar.activation(etb[:, jt * N:(jt + 1) * N], psc[:, :],
                                           ACT.Exp, bias=b_nsh[:, :],
                                           scale=1.0 / math.sqrt(DK))
                if first_exp is None:
                    first_exp = e_i
                # keep the PE clock up while the ACT queue drains the exps
                warm_heavy(
                    2, e_i,
                    scw_al[:, 0:2 * SWP].rearrange("p (s w) -> p s w", s=2)[:, :, 0:SW],
                    etb[:, jt * N:(jt + 1) * N].rearrange("p (s i) -> p s i", s=2))
            # the exp block must come after ALL init sins (one table switch)
            add_dep_helper(first_exp.ins, init_acts[-1].ins, sync=False,
                           reason="group ACT ops by table set")

            # ---------- Kuramoto steps ----------
            gfull = data.tile([P, NT * O], f32)
            gfull3 = gfull[:, :].rearrange("p (t o) -> p t o", o=O)
            rinv = data.tile([P, NT], f32)
            # cvg/svg: cos*g and sin*g, pre-scaled off the critical path
            cvg = data.tile([P, NT * O], f32)
            svg = data.tile([P, NT * O], f32)
            cvg3 = cvg[:, :].rearrange("p (t o) -> p t o", o=O)
            svg3 = svg[:, :].rearrange("p (t o) -> p t o", o=O)
            cnat_l = data.tile([P, HB * O], bf16)
            cnat_h = data.tile([P, HB * O], bf16)
            cnats = [cnat_l, cnat_h]

            phd = data.tile([P, NT * O], f32)
            phd3 = phd[:, :].rearrange("p (t o) -> p t o", o=O)
            phdm = data.tile([P, NT * O], f32)    # phd - pi
            phdm3 = phdm[:, :].rearrange("p (t o) -> p t o", o=O)
            phdmn = data.tile([P, NT * O], f32)   # pi - phd
            phdmn3 = phdmn[:, :].rearrange("p (t o) -> p t o", o=O)
            mhm = work.tile([P, NT * O], f32, tag="mhm")
            mhm3 = mhm[:, :].rearrange("p (t o) -> p t o", o=O)
            # SBUF copy of the transposed coupling sums (GpSimd cannot read
            # PSUM; also shortens the Vector PSUM-port pressure)
            pts = data.tile([P, NT * SW], f32)
            pts3 = pts[:, :].rearrange("p (t w) -> p t w", w=SW)

            def half_update(step, hb, pt3, scw3, scw3_nxt):
                # coupling -> phase update -> next stationary sin/cos
                hs = slice(hb * HB, (hb + 1) * HB)
                nc.vector.tensor_copy(pts3[:, hs, :], pt3[:, hs, :])
                es_v = pts3[:, hs, 0:O]
                ec_v = pts3[:, hs, O:2 * O]
                sv = scw3[hb][:, :, 0:O]
                cv = scw3[hb][:, :, O:2 * O]
                ph_h = ph3[:, hs, :]
                t1 = work.tile([P, HB * O], f32, tag=f"t1{hb}", name=f"t1{hb}")
                t13 = t1[:, :].rearrange("p (t o) -> p t o", o=O)
                t2 = work.tile([P, HB * O], f32, tag=f"t2{hb}", name=f"t2{hb}")
                t23 = t2[:, :].rearrange("p (t o) -> p t o", o=O)
                if step == 0:
                    # g = DT*cs / (E'@1), constant across steps
                    nc.vector.reciprocal(rinv[:, hs, None], pts3[:, hs, 2 * O:SW])
                    nc.vector.tensor_scalar(
                        gfull3[:, hs, :],
                        rinv[:, hs, None].to_broadcast((P, HB, O)),
                        csdt_s[:, :], None, ALU.mult,
                    )
                    nc.vector.tensor_tensor(t13, cv, es_v, ALU.mult)
                    nc.gpsimd.tensor_tensor(t23, sv, ec_v, ALU.mult)
                    nc.vector.tensor_tensor(t13, t13, t23, ALU.subtract)
                    nc.vector.tensor_tensor(t13, t13, gfull3[:, hs, :], ALU.mult)
                else:
                    nc.vector.tensor_tensor(t13, cvg3[:, hs, :], es_v, ALU.mult)
                    nc.gpsimd.tensor_tensor(t23, svg3[:, hs, :], ec_v, ALU.mult)
                    nc.vector.tensor_tensor(t13, t13, t23, ALU.subtract)
                nc.gpsimd.tensor_tensor(mhm3[:, hs, :], phdmn3[:, hs, :], t13,
                                        ALU.subtract)
                nc.vector.tensor_tensor(ph_h, t13, phd3[:, hs, :], ALU.add)
                nc.vector.tensor_tensor(aab3[:, hs, :], t13, phdm3[:, hs, :], ALU.add)
                nc.vector.tensor_tensor(aab3[:, hs, :], aab3[:, hs, :],
                                        mhm3[:, hs, :], ALU.max)
                if step < STEPS - 1:
                    nc.scalar.activation(scw3_nxt[hb][:, :, 0:O], ph_h,
                                         ACT.Sin, bias=b_mpi[:, :], scale=1.0)
                    nc.scalar.activation(scw3_nxt[hb][:, :, O:2 * O], aab3[:, hs, :],
                                         ACT.Sin, bias=b_hpi[:, :], scale=-1.0)
                else:
                    # final sig = cos(phases), per half
                    cn3 = cnats[hb][:, :].rearrange("p (t o) -> p t o", o=O)
                    nc.scalar.activation(cn3, aab3[:, hs, :],
                                         ACT.Sin, bias=b_hpi[:, :], scale=-1.0)

            for step in range(STEPS):
                scw_pair = scws[step % 2]
                scw3 = scw3s[step % 2]
                scw3_nxt = scw3s[(step + 1) % 2]
                last = step == STEPS - 1
                # off-critical-path precompute (overlaps the matmul stream)
                nc.vector.tensor_tensor(phd[:, :], ph[:, :], dtom_s[:, :], ALU.add)
                nc.vector.tensor_scalar(phdm[:, :], phd[:, :], PI, None, ALU.subtract)
                nc.gpsimd.tensor_scalar(phdmn[:, :], phdm[:, :], -1.0, None, ALU.mult)
                if step > 0:
                    for hb in range(2):
                        hs = slice(hb * HB, (hb + 1) * HB)
                        nc.gpsimd.tensor_tensor(cvg3[:, hs, :], scw3[hb][:, :, O:2 * O],
                                                gfull3[:, hs, :], ALU.mult)
                        nc.gpsimd.tensor_tensor(svg3[:, hs, :], scw3[hb][:, :, 0:O],
                                                gfull3[:, hs, :], ALU.mult)

                def dr_pair(pr):
                    # [j_p, 2 j-subtiles, SW] fp8 stationary for DoubleRow
                    t = scw_pair[pr // (HBT // 2)]
                    j = pr % (HBT // 2)
                    return t[:, j * 2 * SWP:(j + 1) * 2 * SWP].rearrange(
                        "p (s w) -> p s w", s=2)[:, :, 0:SW]

                def dr_rhs(pr, ib):
                    # [j_p, 2 j-subtiles, 512] fp8 moving view of E^T
                    return etb[:, pr * 2 * N: (pr + 1) * 2 * N].rearrange(
                        "p (s i) -> p s i", s=2)[:, :, ib * 512:(ib + 1) * 512]

                NPR = NT // 2
                pt = ps1.tile([P, NT * SW], f32, tag="pt", bufs=1)
                pt3 = pt[:, 0:NT * SW].rearrange("p (t w) -> p t w", w=SW)

                # the last step has no next-step matmuls to feed: process the
                # hb1 half FIRST so its (longer) output tail starts earlier
                ib_order = (1, 0) if last else (0, 1)
                pcs = {}
                for ii, ib in enumerate(ib_order):
                    pc = ps2.tile([SW, 512], f32, tag="pc", name=f"pc{ib}")
                    pcs[ib] = pc
                    for pr in range(NPR):
                        nc.tensor.matmul(
                            pc[:, :],
                            lhsT=dr_pair(pr),
                            rhs=dr_rhs(pr, ib),
                            start=(pr == 0),
                            stop=(pr == NPR - 1),
                            perf_mode=DR,
                        )
                        if ii == 1 and pr == 0:
                            hbf = ib_order[0]
                            for itl in range(HB):
                                nc.tensor.transpose(
                                    pt[:, (hbf * HB + itl) * SW:(hbf * HB + itl + 1) * SW],
                                    ce[:, itl * P:(itl + 1) * P],
                                    ident[0:SW, 0:SW],
                                )
                        if ii == 1 and pr == 1:
                            with tc.high_priority(offset=24):
                                half_update(step, ib_order[0], pt3, scw3, scw3_nxt)
                    # PSUM evac split Vector + Scalar (copy is a filler in
                    # every ACT table set: no table switch)
                    ce = work.tile([SW, 512], f32, tag=f"ce{ib}", name=f"ce{ib}")
                    nc.vector.tensor_copy(ce[:, 0:256], pc[:, 0:256])
                    nc.scalar.copy(ce[:, 256:512], pc[:, 256:512])
                hbs = ib_order[1]
                for itl in range(HB):
                    tr = nc.tensor.transpose(
                        pt[:, (hbs * HB + itl) * SW:(hbs * HB + itl + 1) * SW],
                        ce[:, itl * P:(itl + 1) * P],
                        ident[0:SW, 0:SW],
                    )
                # keep the PE boost clock through the chain/ACT window
                if not last:
                    warm_heavy(4, tr, dr_pair(0), dr_rhs(0, 0))
                with tc.high_priority(offset=24):
                    half_update(step, hbs, pt3, scw3, scw3_nxt)

            # ---------- partial output projection (per token-half) ----------
            # sig_h^T tiles [O, 512] via PE transpose, then
            # partial^T[d, i] = sum_o Wo_h.T[o, d] * sig_h^T[o, i].
            identb = const.tile([P, P], bf16)
            nc.vector.tensor_copy(identb[:, :], ident[:, :])
            out_engs = [nc.sync, nc.scalar, nc.gpsimd, nc.sync]
            for oi, hb in enumerate((1, 0)):
                psth = ps2.tile([O, 512], bf16, tag="pc", name=f"psth{hb}")
                for itl in range(HB):
                    nc.tensor.transpose(
                        psth[:, itl * P:(itl + 1) * P],
                        cnats[hb][:, itl * O:(itl + 1) * O],
                        identb[:, :],
                    )
                sgth = work.tile([O, 512], bf16, tag="sgt2", name=f"sgth{hb}")
                nc.vector.tensor_copy(sgth[:, :], psth[:, :])
                for dtile in range(2):
                    po = ps2.tile([P, 512], f32, tag="big", name=f"po{hb}{dtile}")
                    nc.tensor.matmul(
                        po[:, :],
                        lhsT=wob_s[:, dtile * P:(dtile + 1) * P],
                        rhs=sgth[:, :],
                        start=True, stop=True)
                    ot = work.tile([P, 512], bf16, tag="ot", name=f"ot{hb}{dtile}")
                    nc.vector.tensor_copy(ot[:, 0:256], po[:, 0:256])
                    nc.scalar.copy(ot[:, 256:512], po[:, 256:512])
                    out_engs[oi * 2 + dtile].dma_start(
                        outp[dtile * P:(dtile + 1) * P, hb * 512:(hb + 1) * 512],
                        ot[:, :])

    nc.compile()
    return nc


def get_nc():
    if "nc" not in _CACHE:
        _CACHE["nc"] = _build_nc()
    return _CACHE["nc"]


def make_in_maps(x, Wq, Wk, Wp, bp, Wo, bo, omega, coupling_scale):
    import concourse.mybir as mybir

    bf16 = mybir.dt.np(mybir.dt.bfloat16)
    x = np.asarray(x, np.float32)
    Wq = np.asarray(Wq, np.float32)
    Wk = np.asarray(Wk, np.float32)
    Wp = np.asarray(Wp, np.float32)
    bp = np.asarray(bp, np.float32)
    Wo = np.asarray(Wo, np.float32)
    bo = np.asarray(bo, np.float32)
    omega = np.asarray(omega, np.float32)
    cs = float(np.asarray(coupling_scale, np.float32))

    csdt_full = np.full((P, 1), DT * cs, np.float32)

    in_maps = []
    for c in range(NCORES):
        b, h = c // H, c % H
        in_maps.append({
            "xT": np.ascontiguousarray(x[b].T).astype(bf16),
            "wqT": np.ascontiguousarray(Wq[h * DK:(h + 1) * DK, :].T).astype(bf16),
            "wkT": np.ascontiguousarray(Wk[h * DK:(h + 1) * DK, :].T).astype(bf16),
            "wpT": np.ascontiguousarray(Wp[h * O:(h + 1) * O, :].T).astype(bf16),
            "bprt": np.ascontiguousarray(
                np.tile((bp[h * O:(h + 1) * O] + np.pi)[None, :], (P, NT)), np.float32),
            "csdt": csdt_full,
            "dtom": np.ascontiguousarray(
                np.tile((DT * omega[h])[None, :], (P, NT)), np.float32),
            "wob": np.ascontiguousarray(Wo.T[h * O:(h + 1) * O, :]).astype(bf16),
        })
    return in_maps


def run_on_hw(in_maps, trace=False):
    from concourse.bass_utils import run_bass_kernel_spmd

    nc = get_nc()
    return run_bass_kernel_spmd(nc, in_maps, core_ids=list(range(NCORES)), trace=trace)


def assemble(outs, bo):
    """Unshard: sum the 4 per-head partials of each batch, add bo."""
    bo = np.asarray(bo, np.float32)
    full = np.empty((B, N, D), np.float32)
    for b in range(B):
        acc = np.asarray(outs[b * H], np.float32)
        for h in range(1, H):
            acc = acc + np.asarray(outs[b * H + h], np.float32)
        full[b] = acc.T + bo[None, :]
    return full


def kernel(x, Wq, Wk, Wp, bp, Wo, bo, omega, coupling_scale):
    in_maps = make_in_maps(x, Wq, Wk, Wp, bp, Wo, bo, omega, coupling_scale)
    res = run_on_hw(in_maps, trace=False)
    return assemble([res.results[c]["out"] for c in range(NCORES)], bo)


# revision 41
# speedup vs baseline: 1.2460x; 1.0706x over previous
"""AKOrN layer (attention-coupled Kuramoto oscillators) on 8 TRN2 NeuronCores.

Sharding: B*H = 2*4 = 8 (batch, head) pairs -> one pair per core.
Each core computes its head's attention matrix E = exp(scores) entirely in
SBUF (never touches HBM), runs the 5 Kuramoto steps locally, then projects
its own head's contribution to the output: partial^T = Wo_h @ cos(ph)^T,
written transposed ([D, N], bf16). The host unshards by summing the 4
per-head partials of each batch and adding bo - no device collective.

Key performance mechanisms (all HW-measured on this part):
- The PE clock ramps 1.2 -> 2.4 GHz after ~3us of continuous execution and
  drops back after multi-us idles. A low-cost dummy-transpose "keep-warm"
  stream fills every predicted PE idle window so real matmuls run at the
  fast clock.
- The Scalar ACT queue runs [sin-warm | init sins | exp x8 | step sins |
  final cos]: exactly one sin->exp and one exp->sin table switch (hard
  dependency pins the init sins before the first exp).
- HW Sin is only accurate in ~[-pi-0.7, pi+0.7]: phases use the shifted
  representation ph' = phi+pi in [0,2pi), cos(phi) = sin(pi/2-|ph'-pi|)
  with |.| built on the DVE engines (max(x,-x)), keeping Abs off the ACT
  queue. Phases are never wrapped after init: 5 steps drift < 0.7.
- Per-step critical path: ib matmuls -> PSUM evac (Vector+Scalar halves) ->
  PE transposes -> small SBUF copy -> 4-op DVE chain (Vector, with GpSimd
  running the off-path legs) -> s-sin + cos ACTs -> next stationary.

Self-contained: hardcodes all shapes; only imports concourse from the
container's /opt/trn_rl_repo.
"""

import math
import sys

import numpy as np

for _p in ("/opt/trn_rl_repo",):
    if _p not in sys.path:
        sys.path.insert(0, _p)

# Problem constants (from the reference nn.Module)
B, N, D, H, O = 2, 1024, 256, 4, 8
DT, STEPS = 0.1, 5
DK = D // H            # 64 head dim
P = 128                # partitions
NT = N // P            # 8 token tiles
NCORES = 8
SW = 2 * O + 1         # active stationary width per j-tile: [sin | cos | ones] = 17
SWP = 32               # fp8 stationary block stride (dual-fp8 LDWEIGHTS needs 16/32)
ESH = 4.0              # global exp shift: keeps E' = exp(s - ESH) in fp8e4m3
PI = float(np.pi)
TWO_PI = float(2 * np.pi)

_CACHE = {}


def _build_nc():
    import concourse.bacc as bacc
    import concourse.tile as tile
    import concourse.mybir as mybir
    from concourse.masks import make_identity
    from concourse.tile_rust import add_dep_helper

    f32 = mybir.dt.float32
    bf16 = mybir.dt.bfloat16
    f8 = mybir.dt.float8e4
    ALU = mybir.AluOpType
    ACT = mybir.ActivationFunctionType
    DR = mybir.MatmulPerfMode.DoubleRow

    nc = bacc.Bacc(
        "TRN2",
        target_bir_lowering=False,
        debug=False,
        enable_asserts=False,
        num_devices=NCORES,
    )

    # Per-core external inputs (host pre-sliced / transposed)
    xT = nc.dram_tensor("xT", [D, N], bf16, kind="ExternalInput")         # x[b].T (bf16)
    wqT = nc.dram_tensor("wqT", [D, DK], bf16, kind="ExternalInput")      # Wq_h.T
    wkT = nc.dram_tensor("wkT", [D, DK], bf16, kind="ExternalInput")      # Wk_h.T
    wpT = nc.dram_tensor("wpT", [D, O], bf16, kind="ExternalInput")       # Wp_h.T
    bprt = nc.dram_tensor("bprt", [P, NT * O], f32, kind="ExternalInput")  # bp_h+pi tiled
    csdt = nc.dram_tensor("csdt", [P, 1], f32, kind="ExternalInput")      # DT*cs
    dtom = nc.dram_tensor("dtom", [P, NT * O], f32, kind="ExternalInput")  # DT*om tiled
    wob = nc.dram_tensor("wob", [O, D], bf16, kind="ExternalInput")       # Wo.T head rows
    outp = nc.dram_tensor("out", [D, N], bf16, kind="ExternalOutput")     # partial_b,h^T

    with tile.TileContext(nc) as tc:
        with (
            tc.tile_pool(name="const", bufs=1) as const,
            tc.tile_pool(name="data", bufs=1) as data,
            tc.tile_pool(name="work", bufs=2) as work,
            tc.tile_pool(name="ps2", bufs=2, space="PSUM") as ps2,
            tc.tile_pool(name="ps1", bufs=1, space="PSUM") as ps1,
        ):
            # ---------- warm the Sin table set during the input DMA ----------
            with tc.high_priority():
                sin_warm = const.tile([1, 1], f32)
                nc.vector.memset(sin_warm[:, :], 0.0)
                nc.scalar.activation(sin_warm[:, :], sin_warm[:, :], ACT.Sin)

            # ---------- load inputs (spread across engine DMA queues) ----------
            wp_s = const.tile([P, 2 * O], bf16)
            for kt in range(2):
                nc.sync.dma_start(wp_s[:, kt * O:(kt + 1) * O], wpT[kt * P:(kt + 1) * P, :])
            xtb = data.tile([P, 2 * N], bf16)       # x.T, kt-major
            in_engs = [nc.sync, nc.gpsimd]
            for ib in range(2):
                for kt in range(2):
                    in_engs[kt].dma_start(
                        xtb[:, kt * N + ib * 512: kt * N + (ib + 1) * 512],
                        xT[kt * P:(kt + 1) * P, ib * 512:(ib + 1) * 512])

            # identity early: it gates the PE keep-warm stream
            ident = const.tile([P, P], f32)
            with tc.high_priority():
                make_identity(nc, ident[:, :])

            wq_s = const.tile([P, 2 * DK], bf16)
            wk_s = const.tile([P, 2 * DK], bf16)
            for kt in range(2):
                nc.sync.dma_start(wq_s[:, kt * DK:(kt + 1) * DK], wqT[kt * P:(kt + 1) * P, :])
                nc.gpsimd.dma_start(wk_s[:, kt * DK:(kt + 1) * DK], wkT[kt * P:(kt + 1) * P, :])
            bprt_s = const.tile([P, NT * O], f32)
            nc.gpsimd.dma_start(bprt_s[:, :], bprt[:, :])
            csdt_s = const.tile([P, 1], f32)
            nc.gpsimd.dma_start(csdt_s[:, :], csdt[:, :])
            dtom_s = const.tile([P, NT * O], f32)
            nc.gpsimd.dma_start(dtom_s[:, :], dtom[:, :])
            wob_s = const.tile([O, D], bf16)
            nc.gpsimd.dma_start(wob_s[:, :], wob[:, :])

            b_mpi = const.tile([P, 1], f32)
            nc.vector.memset(b_mpi[:, :], -PI)
            b_hpi = const.tile([P, 1], f32)
            nc.vector.memset(b_hpi[:, :], PI / 2)
            b_nsh = const.tile([P, 1], f32)
            nc.vector.memset(b_nsh[:, :], -ESH)

            # ---------- PE keep-warm dummy stream ----------
            # The PE clock needs ~3us of continuous execution to reach
            # 2.4 GHz (HW-measured: 512-col matmuls drop 427ns -> 215ns).
            # These dummy transposes fill predicted PE idle windows so the
            # clock ramps early and never falls back mid-kernel.
            warm_ps = ps1.tile([P, 512], f32, tag="warm", bufs=1)

            def warm(n):
                for _ in range(n):
                    nc.tensor.transpose(
                        warm_ps[:, 0:SW],
                        ident[0:SW, 0:P],
                        ident[0:SW, 0:SW],
                    )

            warm(4)  # cover the input-DMA window

            def warm_heavy(n, after, lhsT, rhs):
                # real-shaped DR matmuls into a scratch bank: heavy enough to
                # hold the PE boost clock, dep-pinned so the list scheduler
                # cannot hoist them out of their intended idle window
                for _ in range(n):
                    wmm = nc.tensor.matmul(
                        warm_ps[0:SW, :], lhsT=lhsT, rhs=rhs,
                        start=True, stop=True, perf_mode=DR)
                    add_dep_helper(wmm.ins, after.ins, sync=False,
                                   reason="PE keep-warm pin")

            # ---------- initial phases, natural layout [i_p, it, O] ----------
            # phi0 = x @ Wp.T directly as 16 small matmuls (xT tiles
            # stationary, Wp moving): no [8,N] evac, no PE transposes.
            php = ps1.tile([P, NT * O], f32, tag="pt", bufs=1)
            with tc.high_priority():
                for it in range(NT):
                    for kt in range(2):
                        nc.tensor.matmul(
                            php[:, it * O:(it + 1) * O],
                            lhsT=xtb[:, kt * N + it * P: kt * N + (it + 1) * P],
                            rhs=wp_s[:, kt * O:(kt + 1) * O],
                            start=(kt == 0),
                            stop=(kt == 1),
                        )
            # ph' = wrap(phi0 + bp + pi) into [0, 2pi): one compare-and-
            # correct wrap (|phi0+bp| < 2pi); never wrapped again (5-step
            # drift < 0.7 stays inside HW Sin's accurate range).
            ph = data.tile([P, NT * O], f32)
            wge = work.tile([P, NT * O], f32, tag="wge")
            aab = work.tile([P, NT * O], f32, tag="aab")
            aab3 = aab[:, :].rearrange("p (t o) -> p t o", o=O)

            with tc.high_priority():
                nc.vector.tensor_tensor(ph[:, :], php[:, :], bprt_s[:, :], ALU.add)
                nc.vector.tensor_scalar(wge[:, :], ph[:, :], TWO_PI, None, ALU.is_ge)
                nc.vector.scalar_tensor_tensor(
                    wge[:, :], ph[:, :], 0.0, wge[:, :], ALU.is_lt, ALU.subtract)
                nc.vector.scalar_tensor_tensor(
                    ph[:, :], wge[:, :], TWO_PI, ph[:, :], ALU.mult, ALU.add)
                # |ph - pi| = max(ph-pi, pi-ph) on the DVE engines
                nc.vector.tensor_scalar(aab[:, :], ph[:, :], PI, None, ALU.subtract)
                nc.vector.tensor_scalar(wge[:, :], aab[:, :], -1.0, None, ALU.mult)
                aab_done = nc.vector.tensor_tensor(aab[:, :], aab[:, :], wge[:, :],
                                                   ALU.max)

            # ---------- stationary sin/cos/ones tiles ----------
            HBT = NT // 2
            scw_al = data.tile([P, HBT * SWP], f8)
            scw_ah = data.tile([P, HBT * SWP], f8)
            scw_bl = data.tile([P, HBT * SWP], f8)
            scw_bh = data.tile([P, HBT * SWP], f8)
            scws = [(scw_al, scw_ah), (scw_bl, scw_bh)]
            scw3s = [tuple(t[:, :].rearrange("p (t w) -> p t w", w=SWP) for t in pair)
                     for pair in scws]
            for pair in scws:
                for t in pair:
                    for jt in range(HBT):
                        nc.gpsimd.memset(t[:, jt * SWP + 2 * O: jt * SWP + SW], 1.0)

            ph3 = ph[:, :].rearrange("p (t o) -> p t o", o=O)
            HB = NT // 2  # it-tiles per half

            # ---------- init sins (before the exps: one exp table load) ----
            init_acts = []
            with tc.high_priority():
                for hb in range(2):
                    hs = slice(hb * HBT, (hb + 1) * HBT)
                    a1 = nc.scalar.activation(scw3s[0][hb][:, :, 0:O], ph3[:, hs, :],
                                              ACT.Sin, bias=b_mpi[:, :], scale=1.0)
                    a2 = nc.scalar.activation(scw3s[0][hb][:, :, O:2 * O], aab3[:, hs, :],
                                              ACT.Sin, bias=b_hpi[:, :], scale=-1.0)
                    init_acts += [a1, a2]

            # ---------- q/k projections (bf16) ----------
            qt = data.tile([DK, N], bf16)
            ktt = data.tile([DK, N], bf16)
            for dst, w_s in ((qt, wq_s), (ktt, wk_s)):
                for ib in range(2):
                    pq = ps2.tile([DK, 512], f32, tag="pc")
                    for kt in range(2):
                        nc.tensor.matmul(
                            pq[:, :],
                            lhsT=w_s[:, kt * DK:(kt + 1) * DK],
                            rhs=xtb[:, kt * N + ib * 512: kt * N + (ib + 1) * 512],
                            start=(kt == 0),
                            stop=(kt == 1),
                        )
                    ev = nc.vector.tensor_copy(dst[:, ib * 512:(ib + 1) * 512], pq[:, :])
                    # keep the Vector queue clear for the init-phase chain
                    # (these evacs gate only the scores, which wait on the
                    # exp pipeline anyway)
                    add_dep_helper(ev.ins, aab_done.ins, sync=False,
                                   reason="init chain before q/k evac")
            # always-ready heavy fillers: the list scheduler emits these into
            # the earliest PE idle gaps (table-load + exp-cascade stalls),
            # bridging the boost clock across the scores phase
            for _ in range(10):
                nc.tensor.matmul(warm_ps[0:SW, :], lhsT=wq_s[:, 0:SW],
                                 rhs=xtb[:, 0:512], start=True, stop=True)

            # ---------- scores + exp -> E^T (fp8, [j_p, jt-major i]) ----------
            etb = data.tile([P, NT * N], f8)
            first_exp = None
            for jt in range(NT):
                psc = ps2.tile([P, N], f32, tag="big")
                for ib in range(2):
                    nc.tensor.matmul(
                        psc[:, ib * 512:(ib + 1) * 512],
                        lhsT=ktt[:, jt * P:(jt + 1) * P],
                        rhs=qt[:, ib * 512:(ib + 1) * 512],
                        start=True,
                        stop=True,
                    )
                # E' = exp(s/sqrt(dk) - ESH): the global shift keeps E' inside
                # fp8e4m3 range; it cancels exactly in coupling = (E'@sc)/(E'@1)
                e_i = nc.scalar.activation(etb[:, jt * N:(jt + 1) * N], psc[:, :],
                                           ACT.Exp, bias=b_nsh[:, :],
                                           scale=1.0 / math.sqrt(DK))
                if first_exp is None:
                    first_exp = e_i
                # keep the PE clock up while the ACT queue drains the exps
                warm_heavy(
                    2, e_i,
                    scw_al[:, 0:2 * SWP].rearrange("p (s w) -> p s w", s=2)[:, :, 0:SW],
                    etb[:, jt * N:(jt + 1) * N].rearrange("p (s i) -> p s i", s=2))
            # the exp block must come after ALL init sins (one table switch)
            add_dep_helper(first_exp.ins, init_acts[-1].ins, sync=False,
                           reason="group ACT ops by table set")

            # ---------- Kuramoto steps ----------
            gfull = data.tile([P, NT * O], f32)
            gfull3 = gfull[:, :].rearrange("p (t o) -> p t o", o=O)
            rinv = data.tile([P, NT], f32)
            # cvg/svg: cos*g and sin*g, pre-scaled off the critical path
            cvg = data.tile([P, NT * O], f32)
            svg = data.tile([P, NT * O], f32)
            cvg3 = cvg[:, :].rearrange("p (t o) -> p t o", o=O)
            svg3 = svg[:, :].rearrange("p (t o) -> p t o", o=O)
            cnat_l = data.tile([P, HB * O], bf16)
            cnat_h = data.tile([P, HB * O], bf16)
            cnats = [cnat_l, cnat_h]

            phd = data.tile([P, NT * O], f32)
            phd3 = phd[:, :].rearrange("p (t o) -> p t o", o=O)
            phdm = data.tile([P, NT * O], f32)    # phd - pi
            phdm3 = phdm[:, :].rearrange("p (t o) -> p t o", o=O)
            phdmn = data.tile([P, NT * O], f32)   # pi - phd
            phdmn3 = phdmn[:, :].rearrange("p (t o) -> p t o", o=O)
            mhm = work.tile([P, NT * O], f32, tag="mhm")
            mhm3 = mhm[:, :].rearrange("p (t o) -> p t o", o=O)
            # SBUF copy of the transposed coupling sums (GpSimd cannot read
            # PSUM; also shortens the Vector PSUM-port pressure)
            pts = data.tile([P, NT * SW], f32)
            pts3 = pts[:, :].rearrange("p (t w) -> p t w", w=SW)

            def half_update(step, hb, pt3, scw3, scw3_nxt):
                # coupling -> phase update -> next stationary sin/cos
                hs = slice(hb * HB, (hb + 1) * HB)
                nc.vector.tensor_copy(pts3[:, hs, :], pt3[:, hs, :])
                es_v = pts3[:, hs, 0:O]
                ec_v = pts3[:, hs, O:2 * O]
                sv = scw3[hb][:, :, 0:O]
                cv = scw3[hb][:, :, O:2 * O]
                ph_h = ph3[:, hs, :]
                t1 = work.tile([P, HB * O], f32, tag=f"t1{hb}", name=f"t1{hb}")
                t13 = t1[:, :].rearrange("p (t o) -> p t o", o=O)
                t2 = work.tile([P, HB * O], f32, tag=f"t2{hb}", name=f"t2{hb}")
                t23 = t2[:, :].rearrange("p (t o) -> p t o", o=O)
                if step == 0:
                    # g = DT*cs / (E'@1), constant across steps
                    nc.vector.reciprocal(rinv[:, hs, None], pts3[:, hs, 2 * O:SW])
                    nc.vector.tensor_scalar(
                        gfull3[:, hs, :],
                        rinv[:, hs, None].to_broadcast((P, HB, O)),
                        csdt_s[:, :], None, ALU.mult,
                    )
                    nc.vector.tensor_tensor(t13, cv, es_v, ALU.mult)
                    nc.gpsimd.tensor_tensor(t23, sv, ec_v, ALU.mult)
                    nc.vector.tensor_tensor(t13, t13, t23, ALU.subtract)
                    nc.vector.tensor_tensor(t13, t13, gfull3[:, hs, :], ALU.mult)
                else:
                    nc.vector.tensor_tensor(t13, cvg3[:, hs, :], es_v, ALU.mult)
                    nc.gpsimd.tensor_tensor(t23, svg3[:, hs, :], ec_v, ALU.mult)
                    nc.vector.tensor_tensor(t13, t13, t23, ALU.subtract)
                nc.gpsimd.tensor_tensor(mhm3[:, hs, :], phdmn3[:, hs, :], t13,
                                        ALU.subtract)
                nc.vector.tensor_tensor(ph_h, t13, phd3[:, hs, :], ALU.add)
                nc.vector.tensor_tensor(aab3[:, hs, :], t13, phdm3[:, hs, :], ALU.add)
                nc.vector.tensor_tensor(aab3[:, hs, :], aab3[:, hs, :],
                                        mhm3[:, hs, :], ALU.max)
                if step < STEPS - 1:
                    nc.scalar.activation(scw3_nxt[hb][:, :, 0:O], ph_h,
                                         ACT.Sin, bias=b_mpi[:, :], scale=1.0)
                    nc.scalar.activation(scw3_nxt[hb][:, :, O:2 * O], aab3[:, hs, :],
                                         ACT.Sin, bias=b_hpi[:, :], scale=-1.0)
                else:
                    # final sig = cos(phases), per half
                    cn3 = cnats[hb][:, :].rearrange("p (t o) -> p t o", o=O)
                    nc.scalar.activation(cn3, aab3[:, hs, :],
                                         ACT.Sin, bias=b_hpi[:, :], scale=-1.0)

            for step in range(STEPS):
                scw_pair = scws[step % 2]
                scw3 = scw3s[step % 2]
                scw3_nxt = scw3s[(step + 1) % 2]
                last = step == STEPS - 1
                # off-critical-path precompute (overlaps the matmul stream)
                nc.vector.tensor_tensor(phd[:, :], ph[:, :], dtom_s[:, :], ALU.add)
                nc.vector.tensor_scalar(phdm[:, :], phd[:, :], PI, None, ALU.subtract)
                nc.gpsimd.tensor_scalar(phdmn[:, :], phdm[:, :], -1.0, None, ALU.mult)
                if step > 0:
                    for hb in range(2):
                        hs = slice(hb * HB, (hb + 1) * HB)
                        nc.gpsimd.tensor_tensor(cvg3[:, hs, :], scw3[hb][:, :, O:2 * O],
                                                gfull3[:, hs, :], ALU.mult)
                        nc.gpsimd.tensor_tensor(svg3[:, hs, :], scw3[hb][:, :, 0:O],
                                                gfull3[:, hs, :], ALU.mult)

                def dr_pair(pr):
                    # [j_p, 2 j-subtiles, SW] fp8 stationary for DoubleRow
                    t = scw_pair[pr // (HBT // 2)]
                    j = pr % (HBT // 2)
                    return t[:, j * 2 * SWP:(j + 1) * 2 * SWP].rearrange(
                        "p (s w) -> p s w", s=2)[:, :, 0:SW]

                def dr_rhs(pr, ib):
                    # [j_p, 2 j-subtiles, 512] fp8 moving view of E^T
                    return etb[:, pr * 2 * N: (pr + 1) * 2 * N].rearrange(
                        "p (s i) -> p s i", s=2)[:, :, ib * 512:(ib + 1) * 512]

                NPR = NT // 2
                pt = ps1.tile([P, NT * SW], f32, tag="pt", bufs=1)
                pt3 = pt[:, 0:NT * SW].rearrange("p (t w) -> p t w", w=SW)

                # the last step has no next-step matmuls to feed: process the
                # hb1 half FIRST so its (longer) output tail starts earlier
                ib_order = (1, 0) if last else (0, 1)
                pcs = {}
                for ii, ib in enumerate(ib_order):
                    pc = ps2.tile([SW, 512], f32, tag="pc", name=f"pc{ib}")
                    pcs[ib] = pc
                    for pr in range(NPR):
                        nc.tensor.matmul(
                            pc[:, :],
                            lhsT=dr_pair(pr),
                            rhs=dr_rhs(pr, ib),
                            start=(pr == 0),
                            stop=(pr == NPR - 1),
                            perf_mode=DR,
                        )
                        if ii == 1 and pr == 0:
                            hbf = ib_order[0]
                            for itl in range(HB):
                                tr_mid = nc.tensor.transpose(
                                    pt[:, (hbf * HB + itl) * SW:(hbf * HB + itl + 1) * SW],
                                    ce[:, itl * P:(itl + 1) * P],
                                    ident[0:SW, 0:SW],
                                )
                            warm_heavy(1, tr_mid, dr_pair(0), dr_rhs(0, 0))
                        if ii == 1 and pr == 1:
                            with tc.high_priority(offset=24):
                                half_update(step, ib_order[0], pt3, scw3, scw3_nxt)
                    # PSUM evac split Vector + Scalar (copy is a filler in
                    # every ACT table set: no table switch)
                    ce = work.tile([SW, 512], f32, tag=f"ce{ib}", name=f"ce{ib}")
                    nc.vector.tensor_copy(ce[:, 0:256], pc[:, 0:256])
                    nc.scalar.copy(ce[:, 256:512], pc[:, 256:512])
                hbs = ib_order[1]
                for itl in range(HB):
                    tr = nc.tensor.transpose(
                        pt[:, (hbs * HB + itl) * SW:(hbs * HB + itl + 1) * SW],
                        ce[:, itl * P:(itl + 1) * P],
                        ident[0:SW, 0:SW],
                    )
                # keep the PE boost clock through the chain/ACT window
                if not last:
                    warm_heavy(6, tr, dr_pair(0), dr_rhs(0, 0))
                with tc.high_priority(offset=24):
                    half_update(step, hbs, pt3, scw3, scw3_nxt)

            # ---------- partial output projection (per token-half) ----------
            # sig_h^T tiles [O, 512] via PE transpose, then
            # partial^T[d, i] = sum_o Wo_h.T[o, d] * sig_h^T[o, i].
            identb = const.tile([P, P], bf16)
            nc.vector.tensor_copy(identb[:, :], ident[:, :])
            out_engs = [nc.sync, nc.scalar, nc.gpsimd, nc.sync]
            for oi, hb in enumerate((1, 0)):
                psth = ps2.tile([O, 512], bf16, tag="pc", name=f"psth{hb}")
                for itl in range(HB):
                    nc.tensor.transpose(
                        psth[:, itl * P:(itl + 1) * P],
                        cnats[hb][:, itl * O:(itl + 1) * O],
                        identb[:, :],
                    )
                sgth = work.tile([O, 512], bf16, tag="sgt2", name=f"sgth{hb}")
                nc.vector.tensor_copy(sgth[:, :], psth[:, :])
                for dtile in range(2):
                    po = ps2.tile([P, 512], f32, tag="big", name=f"po{hb}{dtile}")
                    nc.tensor.matmul(
                        po[:, :],
                        lhsT=wob_s[:, dtile * P:(dtile + 1) * P],
                        rhs=sgth[:, :],
                        start=True, stop=True)
                    ot = work.tile([P, 512], bf16, tag="ot", name=f"ot{hb}{dtile}")
                    nc.vector.tensor_copy(ot[:, 0:256], po[:, 0:256])
                    nc.scalar.copy(ot[:, 256:512], po[:, 256:512])
                    # DMA out per quarter-row chunk as each half-copy lands
                    out_engs[oi * 2 + dtile].dma_start(
                        outp[dtile * P:(dtile + 1) * P,
                             hb * 512:hb * 512 + 256],
                        ot[:, 0:256])
                    out_engs[(oi * 2 + dtile + 1) % 4].dma_start(
                        outp[dtile * P:(dtile + 1) * P,
                             hb * 512 + 256:(hb + 1) * 512],
                        ot[:, 256:512])

    nc.compile()
    return nc


def get_nc():
    if "nc" not in _CACHE:
        _CACHE["nc"] = _build_nc()
    return _CACHE["nc"]


def make_in_maps(x, Wq, Wk, Wp, bp, Wo, bo, omega, coupling_scale):
    import concourse.mybir as mybir

    bf16 = mybir.dt.np(mybir.dt.bfloat16)
    x = np.asarray(x, np.float32)
    Wq = np.asarray(Wq, np.float32)
    Wk = np.asarray(Wk, np.float32)
    Wp = np.asarray(Wp, np.float32)
    bp = np.asarray(bp, np.float32)
    Wo = np.asarray(Wo, np.float32)
    bo = np.asarray(bo, np.float32)
    omega = np.asarray(omega, np.float32)
    cs = float(np.asarray(coupling_scale, np.float32))

    csdt_full = np.full((P, 1), DT * cs, np.float32)

    in_maps = []
    for c in range(NCORES):
        b, h = c // H, c % H
        in_maps.append({
            "xT": np.ascontiguousarray(x[b].T).astype(bf16),
            "wqT": np.ascontiguousarray(Wq[h * DK:(h + 1) * DK, :].T).astype(bf16),
            "wkT": np.ascontiguousarray(Wk[h * DK:(h + 1) * DK, :].T).astype(bf16),
            "wpT": np.ascontiguousarray(Wp[h * O:(h + 1) * O, :].T).astype(bf16),
            "bprt": np.ascontiguousarray(
                np.tile((bp[h * O:(h + 1) * O] + np.pi)[None, :], (P, NT)), np.float32),
            "csdt": csdt_full,
            "dtom": np.ascontiguousarray(
                np.tile((DT * omega[h])[None, :], (P, NT)), np.float32),
            "wob": np.ascontiguousarray(Wo.T[h * O:(h + 1) * O, :]).astype(bf16),
        })
    return in_maps


def run_on_hw(in_maps, trace=False):
    from concourse.bass_utils import run_bass_kernel_spmd

    nc = get_nc()
    return run_bass_kernel_spmd(nc, in_maps, core_ids=list(range(NCORES)), trace=trace)


def assemble(outs, bo):
    """Unshard: sum the 4 per-head partials of each batch, add bo."""
    bo = np.asarray(bo, np.float32)
    full = np.empty((B, N, D), np.float32)
    for b in range(B):
        acc = np.asarray(outs[b * H], np.float32)
        for h in range(1, H):
            acc = acc + np.asarray(outs[b * H + h], np.float32)
        full[b] = acc.T + bo[None, :]
    return full


def kernel(x, Wq, Wk, Wp, bp, Wo, bo, omega, coupling_scale):
    in_maps = make_in_maps(x, Wq, Wk, Wp, bp, Wo, bo, omega, coupling_scale)
    res = run_on_hw(in_maps, trace=False)
    return assemble([res.results[c]["out"] for c in range(NCORES)], bo)
